# revision 59
# baseline (speedup 1.0000x reference)
"""Trainium2 Bass kernel for nn_CopresheafLayer (GNN message passing).

Math (reference):
    h     = silu(rbf @ f1_w.T + f1_b)                    # [E, 128]
    phi   = (h @ f2_w.T + f2_b) * envelope[:, None]      # [E, 64]
    msg   = (phi * (x[src] @ W_send.T)) @ W_recv         # [E, 128]
    agg   = segment_sum(msg, tgt, N)                     # [N, 128]
    gate  = silu(agg @ g1_w.T + g1_b) @ g2_w.T + g2_b
    y     = LayerNorm(x + gate) * ln_g + ln_b

Design (all indexing work on the host, all FLOPs on device):
  * Nodes are permuted onto (core, 128-target-window) buckets by an LPT
    greedy over indegree, equalizing edge counts; all 8 cores then share
    one program with T = sum_w ceil(maxcore cnt[w]/128) scatter tiles.
  * Host pre-gathers x[src] (pure indexing) into slot order, transposed
    (xeT [128, T*128] bf16), and pre-builds the envelope-weighted scatter
    one-hot Sd [128 edge-slot, T*128] bf16 -- both streamed as linear DMA;
    no device gather (SWDGE descriptor generation was the v1 bottleneck).
  * Per batch of 4 tiles (512 edges):
      xs_j  = xe_j  @ wsendT -> pcomb[:, j, 64:128]  (PSUM f32)
      h     = f1wT-stationary matmul over rbfT [32, 512], ACT silu -> bf16
      phi_j = h_j @ f2wT     -> pcomb[:, j, 0:64]
      DVE: cast xs half to bf16, then product phi*xs (phi from PSUM)
      psc[:, w] += matmul(lhsT=phixs_j, rhs=S_j)     (PSUM accumulate);
      scatters are emitted one batch late so the in-order PE queue never
      stalls on the DVE product.
  * f2_b folded via augmented W_recv rows 64:128 (raw xs scattered too);
    envelope lives in Sd, so it multiplies both cat halves exactly once.
  * Tail per 4-window block: W_recv-aug and g1 matmuls run 512 wide with
    constant stationaries; per-window g2 matmul + residual; LayerNorm
    stats on DVE with one batched ACT Sqrt per block (2 table loads per
    block instead of 2 per window).
"""

import math
import os
import sys

import numpy as np

sys.path.insert(0, "/opt/trn_rl_repo")

P = 128   # SBUF partitions / d_node
D = 128   # d_node
DS = 64   # d_stalk
R = 32    # n_rbf
TB = 4    # tiles per batch (512 edges)
WB = 4    # windows per scatter PSUM block

N_NODES = 100000
N_CORES = 8


# --------------------------------------------------------------------------
# Host-side preprocessing (index manipulation only -- no model FLOPs)
# --------------------------------------------------------------------------

def _preprocess(x, edge_index, rbf, envelope, W_send, W_recv, f1_w, f1_b,
                f2_w, f2_b, g1_w, g1_b, g2_w, g2_b, ln_g, ln_b, n_cores):
    import ml_dtypes

    bf16 = ml_dtypes.bfloat16
    N = x.shape[0]
    E = edge_index.shape[1]
    assert N % n_cores == 0
    NS = N // n_cores                 # nodes per core
    NW = (NS + P - 1) // P            # 128-node windows per core
    last_nw = NS - P * (NW - 1)

    src = np.asarray(edge_index[0]).astype(np.int64)
    tgt = np.asarray(edge_index[1]).astype(np.int64)
    env = np.asarray(envelope, dtype=np.float32)
    rbf = np.asarray(rbf, dtype=np.float32)
    x_f = np.asarray(x, dtype=np.float32)
    x_bf = x_f.astype(bf16)

    # --- degree-balanced node placement (LPT greedy over target nodes) ---
    # Targets are assigned to (core, window) buckets so per-bucket indegree
    # sums equalize; this minimizes the shared tile count T = sum_w
    # ceil(max_core cnt[c,w] / 128) and balances work across cores.
    import heapq

    n_buckets = n_cores * NW
    cap = np.full(n_buckets, P, dtype=np.int64)
    cap[NW - 1::NW] = NS - P * (NW - 1)   # short last window per core
    deg = np.bincount(tgt, minlength=N)
    node_order = np.argsort(-deg, kind="stable")
    heap = [(0, b) for b in range(n_buckets)]
    heapq.heapify(heap)
    fill = np.zeros(n_buckets, dtype=np.int64)
    newpos = np.empty(N, dtype=np.int64)
    for n in node_order:
        s, b = heapq.heappop(heap)
        c, w = divmod(b, NW)
        newpos[n] = c * NS + w * P + fill[b]
        fill[b] += 1
        if fill[b] < cap[b]:
            heapq.heappush(heap, (s + int(deg[n]), b))
    orig_of = np.empty(N, dtype=np.int64)
    orig_of[newpos] = np.arange(N)

    tgt = newpos[tgt]                    # edges now address permuted targets
    order = np.argsort(tgt, kind="stable")
    tgt_s, src_s, env_s = tgt[order], src[order], env[order]
    core_of = tgt_s // NS
    win_s = (tgt_s % NS) // P

    # shared group structure: tiles per window = ceil(max-core count / 128)
    cnt = np.zeros((n_cores, NW), dtype=np.int64)
    np.add.at(cnt, (core_of, win_s), 1)
    G = np.maximum(-(-cnt.max(axis=0) // P), 1)   # >=1 tile per window
    grp_start = np.concatenate([[0], np.cumsum(G)[:-1]])
    T = int(G.sum())
    pad = (-T) % (2 * TB)             # T multiple of 2*TB; pad -> last win
    win_first = grp_start.copy()
    win_last = grp_start + G - 1
    tile_win = np.repeat(np.arange(NW), G)
    if pad:
        tile_win = np.concatenate([tile_win, np.full(pad, NW - 1)])
        win_last[NW - 1] += pad
        T += pad

    per_core = []
    for c in range(n_cores):
        slots = T * P
        src_slot = np.zeros(slots, dtype=np.int64)
        env_slot = np.zeros(slots, dtype=np.float32)
        tgtl_slot = np.zeros(slots, dtype=np.float32)
        sel = core_of == c
        tw = win_s[sel]
        # position within the window group (vectorized cumcount)
        pos = np.zeros(len(tw), dtype=np.int64)
        so = np.argsort(tw, kind="stable")
        gs = tw[so]
        pos[so] = np.arange(len(gs)) - np.searchsorted(gs, gs)
        slot = grp_start[tw] * P + pos
        src_slot[slot] = src_s[sel]
        env_slot[slot] = env_s[sel]
        tgtl_slot[slot] = tgt_s[sel] - c * NS - tw * P
        rbf_slot = np.zeros((slots, R), dtype=np.float32)
        rbf_slot[slot] = rbf[order[sel]]

        # host-built scatter one-hot (pure indexing): S[t, p, c] = env of
        # slot (t,p) iff its window-local target is c.  Streamed as the
        # matmul rhs, killing the on-device S build entirely.
        S_arr = np.zeros((T, P, P), dtype=bf16)
        tl = tgtl_slot.astype(np.int64)
        S_arr[np.arange(slots) // P, np.arange(slots) % P, tl] = \
            env_slot.astype(bf16)

        maps = {
            # host gather of x rows (pure indexing), transposed for lhsT use
            "xeT": np.ascontiguousarray(x_bf[src_slot].T),
            "rbfT": np.ascontiguousarray(rbf_slot.T.astype(bf16)),
            "Sd": np.ascontiguousarray(
                S_arr.transpose(1, 0, 2).reshape(P, T * P)),
            "xres": np.ascontiguousarray(
                x_f[orig_of[c * NS:(c + 1) * NS]]
                + np.asarray(g2_b, np.float32)[None, :]),
        }
        per_core.append(maps)

    f2b = np.asarray(f2_b, np.float32)
    wrecv = np.asarray(W_recv, np.float32)
    shared = {
        "f1wT": np.ascontiguousarray(np.asarray(f1_w, np.float32).T.astype(bf16)),
        "f2wT": np.ascontiguousarray(np.asarray(f2_w, np.float32).T.astype(bf16)),
        "wsendT": np.ascontiguousarray(np.asarray(W_send, np.float32).T.astype(bf16)),
        "wrecvaug": np.ascontiguousarray(
            np.vstack([wrecv, f2b[:, None] * wrecv]).astype(bf16)),
        "g1wT": np.ascontiguousarray(np.asarray(g1_w, np.float32).T.astype(bf16)),
        "g2wT": np.ascontiguousarray(np.asarray(g2_w, np.float32).T.astype(bf16)),
        "f1b": np.asarray(f1_b, np.float32).reshape(D, 1).copy(),
        "g1b": np.asarray(g1_b, np.float32).reshape(D, 1).copy(),
    }
    ln_g = np.asarray(ln_g, np.float32)
    ln_b = np.asarray(ln_b, np.float32)
    ln_trivial = bool(np.all(ln_g == 1.0) and np.all(ln_b == 0.0))
    if not ln_trivial:
        shared["lngrow"] = ln_g.reshape(1, D).copy()
        shared["lnbrow"] = ln_b.reshape(1, D).copy()

    cfg = dict(N=N, E=E, NS=NS, NW=NW, T=T, last_nw=last_nw,
               win_first=win_first.tolist(), win_last=win_last.tolist(),
               tile_win=[int(v) for v in tile_win], ln_trivial=ln_trivial,
               n_cores=n_cores)
    return cfg, shared, per_core, orig_of


# --------------------------------------------------------------------------
# Bass program (identical across cores)
# --------------------------------------------------------------------------

def _build_program(cfg):
    from contextlib import ExitStack

    import concourse.bacc as bacc
    import concourse.bass as bass
    import concourse.tile as tile
    from concourse import mybir

    f32 = mybir.dt.float32
    bf = mybir.dt.bfloat16
    Alu = mybir.AluOpType
    Act = mybir.ActivationFunctionType

    NS, NW, T = cfg["NS"], cfg["NW"], cfg["T"]
    win_first, win_last = cfg["win_first"], cfg["win_last"]
    tile_win = cfg["tile_win"]
    last_nw = cfg["last_nw"]

    nc = bacc.Bacc("TRN2", target_bir_lowering=False, debug=False)

    xeT_d = nc.dram_tensor("xeT", [P, T * P], bf, kind="ExternalInput")
    rbfT_d = nc.dram_tensor("rbfT", [R, T * P], bf, kind="ExternalInput")
    Sd_d = nc.dram_tensor("Sd", [P, T * P], bf, kind="ExternalInput")
    xres_d = nc.dram_tensor("xres", [NS, D], f32, kind="ExternalInput")
    f1wT_d = nc.dram_tensor("f1wT", [R, D], bf, kind="ExternalInput")
    f2wT_d = nc.dram_tensor("f2wT", [D, DS], bf, kind="ExternalInput")
    wsendT_d = nc.dram_tensor("wsendT", [D, DS], bf, kind="ExternalInput")
    wrecvaug_d = nc.dram_tensor("wrecvaug", [P, D], bf, kind="ExternalInput")
    g1wT_d = nc.dram_tensor("g1wT", [D, D], bf, kind="ExternalInput")
    g2wT_d = nc.dram_tensor("g2wT", [D, D], bf, kind="ExternalInput")
    f1b_d = nc.dram_tensor("f1b", [D, 1], f32, kind="ExternalInput")
    g1b_d = nc.dram_tensor("g1b", [D, 1], f32, kind="ExternalInput")
    if not cfg["ln_trivial"]:
        lngrow_d = nc.dram_tensor("lngrow", [1, D], f32, kind="ExternalInput")
        lnbrow_d = nc.dram_tensor("lnbrow", [1, D], f32, kind="ExternalInput")
    y_d = nc.dram_tensor("y", [NS, D], f32, kind="ExternalOutput")

    with ExitStack() as ctx:
        tc = ctx.enter_context(tile.TileContext(nc))
        consts = ctx.enter_context(tc.tile_pool(name="consts", bufs=1))

        def load_const(dt_tensor, shape, dtype, name):
            t = consts.tile(shape, dtype, name=name)
            nc.sync.dma_start(out=t, in_=dt_tensor[:, :])
            return t

        f1wT_sb = load_const(f1wT_d, [R, D], bf, "f1wT_sb")
        f2wT_sb = load_const(f2wT_d, [D, DS], bf, "f2wT_sb")
        wsendT_sb = load_const(wsendT_d, [D, DS], bf, "wsendT_sb")
        wrecvaug_sb = load_const(wrecvaug_d, [P, D], bf, "wrecvaug_sb")
        g1wT_sb = load_const(g1wT_d, [D, D], bf, "g1wT_sb")
        g2wT_sb = load_const(g2wT_d, [D, D], bf, "g2wT_sb")
        f1b_sb = load_const(f1b_d, [D, 1], f32, "f1b_sb")
        g1b_sb = load_const(g1b_d, [D, 1], f32, "g1b_sb")
        eps_sb = consts.tile([P, 1], f32, name="eps_sb")
        nc.vector.memset(eps_sb, 1e-5)
        if not cfg["ln_trivial"]:
            lng_bc = consts.tile([P, D], f32, name="lng_bc")
            nc.sync.dma_start(
                out=lng_bc,
                in_=bass.AP(tensor=lngrow_d, offset=0, ap=[[0, P], [1, D]]))
            lnb_bc = consts.tile([P, D], f32, name="lnb_bc")
            nc.sync.dma_start(
                out=lnb_bc,
                in_=bass.AP(tensor=lnbrow_d, offset=0, ap=[[0, P], [1, D]]))

        xep = ctx.enter_context(tc.tile_pool(name="xep", bufs=6))
        rbfp = ctx.enter_context(tc.tile_pool(name="rbfp", bufs=6))
        hp = ctx.enter_context(tc.tile_pool(name="hp", bufs=3))
        pxp = ctx.enter_context(tc.tile_pool(name="pxp", bufs=4))
        sp = ctx.enter_context(tc.tile_pool(name="sp", bufs=6))
        tailp = ctx.enter_context(tc.tile_pool(name="tailp", bufs=2))
        ublkp = ctx.enter_context(tc.tile_pool(name="ublkp", bufs=2))
        hps = ctx.enter_context(tc.tile_pool(name="hps", bufs=2, space="PSUM"))
        cps = ctx.enter_context(tc.tile_pool(name="cps", bufs=3, space="PSUM"))
        scps = ctx.enter_context(tc.tile_pool(name="scps", bufs=2, space="PSUM"))
        tps = ctx.enter_context(tc.tile_pool(name="tps", bufs=1, space="PSUM"))

        # tail, batched per WB-window block: the W_recv-aug and g1 matmuls
        # run 512-wide with constant stationaries; only the g2 matmul (whose
        # stationary is the per-window silu output) stays per-window.
        # Batching the single ACT Sqrt keeps table swaps to 2 per block.
        # tail is staged across batch iterations: each stage ends right
        # before a PE instruction that waits on an ACT result, so a full
        # batch of independent matmuls separates the stages in the
        # in-order PE queue (no PE stall on the ACT copies/silu).
        def tail_stage1(blk, ps_sc):
            nwb = min(WB, NW - blk * WB)
            nF = nwb * P
            sc_sb = tailp.tile([P, WB * P], bf, tag="sc")
            nc.scalar.activation(out=sc_sb[:, :nF], in_=ps_sc[:, :nF],
                                 func=Act.Copy)
            pB = tps.tile([P, WB * P], f32, tag="tp")
            nc.tensor.matmul(out=pB[:, :nF], lhsT=wrecvaug_sb,
                             rhs=sc_sb[:, :nF], start=True, stop=True)
            B_sb = tailp.tile([P, WB * P], bf, tag="B")
            nc.scalar.activation(out=B_sb[:, :nF], in_=pB[:, :nF],
                                 func=Act.Copy)
            return (blk, nwb, nF, B_sb)

        def tail_stage2(st):
            blk, nwb, nF, B_sb = st
            pC = tps.tile([P, WB * P], f32, tag="tp")
            nc.tensor.matmul(out=pC[:, :nF], lhsT=g1wT_sb, rhs=B_sb[:, :nF],
                             start=True, stop=True)
            C_sb = tailp.tile([P, WB * P], bf, tag="C")
            nc.scalar.activation(out=C_sb[:, :nF], in_=pC[:, :nF],
                                 func=Act.Silu, bias=g1b_sb[:, 0:1], scale=1.0)
            return (blk, nwb, C_sb)

        def tail_stage3(st):
            blk, nwb, C_sb = st
            ublk = ublkp.tile([P, WB, D], f32, tag="ublk",
                              name=f"ublk{blk % 2}")
            w0 = blk * WB
            for k in range(nwb):
                w = w0 + k
                nwn = P if w < NW - 1 else last_nw
                xw = tailp.tile([P, D], f32, tag="xw")
                nc.gpsimd.dma_start(out=xw[:nwn, :],
                                    in_=xres_d[w * P:w * P + nwn, :])
                pG = tps.tile([P, WB * P], f32, tag="tp")
                nc.tensor.matmul(out=pG[:, 0:P],
                                 lhsT=C_sb[:, k * P:(k + 1) * P],
                                 rhs=g2wT_sb, start=True, stop=True)
                nc.vector.scalar_tensor_tensor(
                    out=ublk[:, k, :], in0=pG[:, 0:P], scalar=1.0, in1=xw,
                    op0=Alu.mult, op1=Alu.add)
            st = tailp.tile([P, WB, 6], f32, tag="st")
            mv = tailp.tile([P, WB, 2], f32, tag="mv")
            for k in range(nwb):
                nc.vector.bn_stats(out=st[:, k, :], in_=ublk[:, k, :])
                nc.vector.bn_aggr(out=mv[:, k, :], in_=st[:, k, :])
            sd = tailp.tile([P, WB, 1], f32, tag="sd")
            nc.scalar.activation(out=sd[:, :nwb, :], in_=mv[:, :nwb, 1:2],
                                 func=Act.Sqrt, bias=eps_sb[:, 0:1], scale=1.0)
            rs = tailp.tile([P, WB, 1], f32, tag="rs")
            nc.vector.reciprocal(out=rs[:, :nwb, :], in_=sd[:, :nwb, :])
            for k in range(nwb):
                w = w0 + k
                nwn = P if w < NW - 1 else last_nw
                v = tailp.tile([P, D], f32, tag="v")
                nc.vector.tensor_scalar(
                    out=v, in0=ublk[:, k, :], scalar1=mv[:, k, 0:1],
                    scalar2=rs[:, k, 0:1], op0=Alu.subtract, op1=Alu.mult)
                if not cfg["ln_trivial"]:
                    v2 = tailp.tile([P, D], f32, tag="v2")
                    nc.vector.tensor_tensor(out=v2, in0=v, in1=lng_bc,
                                            op=Alu.mult)
                    v3 = tailp.tile([P, D], f32, tag="v3")
                    nc.vector.tensor_tensor(out=v3, in0=v2, in1=lnb_bc,
                                            op=Alu.add)
                    v = v3
                nc.sync.dma_start(out=y_d[w * P:w * P + nwn, :],
                                  in_=v[:nwn, :])

        psc = {}       # block -> psum tile [P, WB*P]
        tail_jobs = []  # [(stage, state)] advanced one stage per batch

        def pump_tail():
            if not tail_jobs:
                return
            stage, st = tail_jobs.pop(0)
            if stage == 1:
                tail_jobs.append((2, tail_stage1(*st)))
            elif stage == 2:
                tail_jobs.append((3, tail_stage2(st)))
            else:
                tail_stage3(st)

        def emit_scatters(phixs_p, S_p, t0p):
            for j in range(TB):
                t = t0p + j
                w = tile_win[t]
                blk = w // WB
                if blk not in psc:
                    psc[blk] = scps.tile([P, WB * P], f32, tag="sc",
                                         name=f"psc{blk}")
                wi = w % WB
                nc.tensor.matmul(out=psc[blk][:, wi * P:(wi + 1) * P],
                                 lhsT=phixs_p[:, j, :],
                                 rhs=S_p[:, j * P:(j + 1) * P],
                                 start=(t == win_first[w]),
                                 stop=(t == win_last[w]),
                                 skip_group_check=True)
                if t == win_last[w] and (w % WB == WB - 1 or w == NW - 1):
                    tail_jobs.append((1, (blk, psc[blk])))
                    psc.pop(blk, None)

        pending = None     # (phixs, S_sb4, t0) of the previous batch
        for b in range(T // TB):
            t0 = b * TB
            # stream edge operands two batches per DMA, issued on the Pool
            # engine's queue (Sync's DMA_DIRECT2D dispatch is a bottleneck)
            if b % 2 == 0:
                xe2 = xep.tile([P, 2 * TB * P], bf, tag="xe")
                nc.gpsimd.dma_start(
                    out=xe2, in_=xeT_d[:, t0 * P:(t0 + 2 * TB) * P])
                rbf2 = rbfp.tile([R, 2 * TB * P], bf, tag="rbf")
                nc.sync.dma_start(
                    out=rbf2, in_=rbfT_d[:, t0 * P:(t0 + 2 * TB) * P])
                S2 = sp.tile([P, 2 * TB * P], bf, tag="S2")
                nc.sync.dma_start(
                    out=S2, in_=Sd_d[:, t0 * P:(t0 + 2 * TB) * P])
            half = (b % 2) * TB * P
            xe_sb = xe2[:, half:half + TB * P]
            rbf_sb = rbf2[:, half:half + TB * P]
            S_sb4 = S2[:, half:half + TB * P]

            pcomb = cps.tile([P, TB * P], f32, tag="pc")
            pc3 = pcomb.rearrange("p (j c) -> p j c", c=P)
            # xs_j first (only needs the DMA), so the ACT copy can overlap
            # with the h -> silu -> phi chain
            for j in range(TB):
                nc.tensor.matmul(out=pcomb[:, j * P + DS:(j + 1) * P],
                                 lhsT=xe_sb[:, j * P:(j + 1) * P],
                                 rhs=wsendT_sb, start=True, stop=True)
            phixs = pxp.tile([P, TB, P], bf, tag="px")
            nc.vector.tensor_copy(out=phixs[:, :, DS:P], in_=pc3[:, :, DS:P])

            # h = silu(rbf @ f1 + b): one 512-wide matmul, f1wT stationary
            ph = hps.tile([P, TB * P], f32, tag="h")
            nc.tensor.matmul(out=ph, lhsT=f1wT_sb, rhs=rbf_sb,
                             start=True, stop=True)
            h_sb = hp.tile([P, TB * P], bf, tag="h_sb")
            nc.scalar.activation(out=h_sb, in_=ph, func=Act.Silu,
                                 bias=f1b_sb[:, 0:1], scale=1.0)
            for j in range(TB):
                nc.tensor.matmul(out=pcomb[:, j * P:j * P + DS],
                                 lhsT=h_sb[:, j * P:(j + 1) * P],
                                 rhs=f2wT_sb, start=True, stop=True)
            # product phi*xs: phi read straight from PSUM, result to bf16
            nc.vector.tensor_tensor(
                out=phixs[:, :, 0:DS], in0=pc3[:, :, 0:DS],
                in1=phixs[:, :, DS:P], op=Alu.mult)

            # scatter the PREVIOUS batch: its product is long done, so the
            # in-order PE queue never stalls waiting on the DVE multiply
            pump_tail()
            if pending is not None:
                emit_scatters(*pending)
            pending = (phixs, S_sb4, t0)
        emit_scatters(*pending)
        while tail_jobs:
            pump_tail()

    nc.compile()
    return nc


# --------------------------------------------------------------------------
# Entry point
# --------------------------------------------------------------------------

def _run(inputs, trace=False, n_cores=N_CORES):
    import time as _time
    t0 = _time.time()
    cfg, shared, per_core, orig_of = _preprocess(n_cores=n_cores, **inputs)
    t1 = _time.time()
    nc = _build_program(cfg)
    t2 = _time.time()

    from concourse.bass_utils import run_bass_kernel_spmd
    from concourse.compiler_utils import get_compiler_flags, set_compiler_flags

    if os.environ.get("LDW_OPT", "0") == "1":
        flags = [f.replace("--enable-ldw-opt=false", "--enable-ldw-opt=true")
                 for f in get_compiler_flags()]
        set_compiler_flags(flags)

    in_maps = []
    for c in range(n_cores):
        m = dict(shared)
        m.update(per_core[c])
        in_maps.append(m)
    res = run_bass_kernel_spmd(nc, in_maps, core_ids=list(range(n_cores)),
                               trace=trace)
    t3 = _time.time()
    print(f"[kernel] preprocess {t1 - t0:.1f}s  build {t2 - t1:.1f}s  "
          f"compile+run {t3 - t2:.1f}s", file=sys.stderr)
    y_cat = np.concatenate([res.results[c]["y"] for c in range(n_cores)],
                           axis=0)[:cfg["N"]]
    out = np.empty_like(y_cat, dtype=np.float32)
    out[orig_of] = y_cat           # undo the degree-balancing permutation
    return out, res


def kernel(**inputs):
    return _run(inputs)[0]
